# revision 4
# baseline (speedup 1.0000x reference)
"""Trainium2 Bass kernel for nn_Attention_Block (quirky reshape + axis-2 softmax).

Reference math (B=4, T=2048, D=512, H=8, hd=64):
  q = x @ Wq.T ; k = x @ Wk.T ; v = x @ Wv.T          (per batch, [T, D])
  q/k/v reshaped RAW to [H, T, hd]  -> head h == contiguous 256-row chunk of
  the [T, D] matrix, reinterpreted as [2048, 64].
  scores = q~ @ k~.T / 8 ; attn = softmax(scores, axis=q) ; out = attn @ v~
  reshaped back, then @ Wo.T + bo.

Because the head split is a raw reshape, the whole problem decomposes into
B*H = 32 independent 256-row units.  We run 8-way data parallel (4 units per
core, weights replicated, no collectives).

Per-unit kernel layout (core insight: with S^T = k~ @ q~.T the softmax over q
becomes a row softmax along the free axis):
  - permuted ordering q' = (j, r): q~'^T block j = rows 64j..64j+63 of
    QT = Wq @ x_u^T  (so QT tiles natively hold two blocks per 128 partitions)
  - QT replicated on both partition halves so S^T matmuls can row-tile 2x
  - exp on ScalarE with accum_out giving Z (no max subtraction needed:
    |scores/8| < ~1.2 for these inputs), 1/Z folded into v
  - PV col-tiled 2x producing out^T directly in the layout that makes
    OC^T (the final-projection lhsT) a set of aligned psum->sbuf copies.
"""

import numpy as np

D = 512
TCORE = 1024  # rows of x per core
NU = 4        # units (b,h pairs) per core
NCORES = 8

_CHUNK_ORDER = [
    (0, 0), (1, 0), (0, 1), (1, 1),
    (2, 0), (3, 0), (2, 1), (3, 1),
    (4, 0), (5, 0), (4, 1), (5, 1),
    (6, 0), (7, 0), (6, 1), (7, 1),
]

_nc_cache = None


def _build_nc():
    from contextlib import ExitStack

    import concourse.bass as bass
    import concourse.bacc as bacc
    import concourse.mybir as mybir
    import concourse.tile as tile
    from concourse.masks import make_identity

    F32 = mybir.dt.float32
    EXP = mybir.ActivationFunctionType.Exp

    nc = bacc.Bacc()
    x_d = nc.dram_tensor("x", [TCORE, D], F32, kind="ExternalInput")
    w_d = {
        nm: nc.dram_tensor(nm, [D, D], F32, kind="ExternalInput")
        for nm in ("Wq", "Wk", "Wv", "Wo")
    }
    bo_d = nc.dram_tensor("bo", [D], F32, kind="ExternalInput")
    out_d = nc.dram_tensor("out", [TCORE, D], F32, kind="ExternalOutput")

    with tile.TileContext(nc) as tc, ExitStack() as ctx:
        const = ctx.enter_context(tc.tile_pool(name="const", bufs=1))
        wload = ctx.enter_context(tc.tile_pool(name="wload", bufs=2))
        unitp = ctx.enter_context(tc.tile_pool(name="unitp", bufs=2))
        ppool = ctx.enter_context(tc.tile_pool(name="ppool", bufs=3))
        stats = ctx.enter_context(tc.tile_pool(name="stats", bufs=8))
        outp = ctx.enter_context(tc.tile_pool(name="outp", bufs=3))
        ps_s = ctx.enter_context(tc.tile_pool(name="ps_s", bufs=2, space="PSUM"))
        ps_o = ctx.enter_context(tc.tile_pool(name="ps_o", bufs=1, space="PSUM"))
        ps_m = ctx.enter_context(tc.tile_pool(name="ps_m", bufs=2, space="PSUM"))

        ident = const.tile([128, 128], F32, tag="ident")
        make_identity(nc, ident)

        # ---- x load + on-chip transpose: xT[p, k, c] = x[c, 128k+p]
        x_sb = const.tile([128, 8, D], F32, tag="x_sb")
        nc.sync.dma_start(out=x_sb, in_=x_d[:, :].rearrange("(t p) d -> p t d", p=128))
        xT = const.tile([128, 4, TCORE], F32, tag="xT")
        for t in range(8):
            for k in range(4):
                pst = ps_m.tile([128, 512], F32, tag="misc")
                nc.tensor.transpose(pst[:, 0:128], x_sb[:, t, 128 * k:128 * k + 128], ident)
                nc.vector.tensor_copy(out=xT[:, k, 128 * t:128 * t + 128], in_=pst[:, 0:128])

        # ---- weight loads + transposes: wT[p, k, c] = W[c, 128k+p] (= W^T row-major)
        wT = {}
        for nm in ("Wq", "Wk", "Wv", "Wo"):
            w_sb = wload.tile([128, 4, D], F32, tag="w_sb")
            nc.sync.dma_start(
                out=w_sb, in_=w_d[nm][:, :].rearrange("(t p) d -> p t d", p=128)
            )
            wt = const.tile([128, 4, D], F32, tag=f"{nm}T")
            for t in range(4):
                for k in range(4):
                    pst = ps_m.tile([128, 512], F32, tag="misc")
                    nc.tensor.transpose(
                        pst[:, 0:128], w_sb[:, t, 128 * k:128 * k + 128], ident
                    )
                    nc.vector.tensor_copy(
                        out=wt[:, k, 128 * t:128 * t + 128], in_=pst[:, 0:128]
                    )
            wT[nm] = wt

        bo_bc = const.tile([128, D], F32, tag="bo")
        nc.sync.dma_start(
            out=bo_bc, in_=bass.AP(tensor=bo_d, offset=0, ap=[[0, 128], [1, D]])
        )

        for u in range(NU):
            xTu = 256 * u

            # ---- QT = Wq @ x_u^T, block j replicated on both partition halves
            qt = unitp.tile([128, 8, 256], F32, tag="qt")
            for j in range(8):
                psq = ps_m.tile([128, 512], F32, tag="misc")
                for ki in range(4):
                    st, sp = (ki == 0), (ki == 3)
                    lhs = wT["Wq"][:, ki, 64 * j:64 * j + 64]
                    rhs = xT[:, ki, xTu:xTu + 256]
                    nc.tensor.matmul(
                        psq[0:64, 0:256], lhsT=lhs, rhs=rhs, start=st, stop=sp,
                        tile_position=(0, 0),
                    )
                    nc.tensor.matmul(
                        psq[64:128, 0:256], lhsT=lhs, rhs=rhs, start=st, stop=sp,
                        tile_position=(0, 64),
                    )
                nc.vector.tensor_copy(out=qt[:, j, :], in_=psq[:, 0:256])

            # ---- KT = Wk @ x_u^T, natural layout (block 2m | 2m+1 per tile)
            kt = unitp.tile([128, 4, 256], F32, tag="kt")
            for mt in range(4):
                psk = ps_m.tile([128, 512], F32, tag="misc")
                for ki in range(4):
                    nc.tensor.matmul(
                        psk[:, 0:256],
                        lhsT=wT["Wk"][:, ki, 128 * mt:128 * mt + 128],
                        rhs=xT[:, ki, xTu:xTu + 256],
                        start=(ki == 0), stop=(ki == 3),
                    )
                nc.vector.tensor_copy(out=kt[:, mt, :], in_=psk[:, 0:256])

            # ---- V = x_u @ Wv^T, natural layout
            vv = unitp.tile([128, 2, 512], F32, tag="vv")
            for nt in range(2):
                psv = ps_m.tile([128, 512], F32, tag="misc")
                for ki in range(4):
                    nc.tensor.matmul(
                        psv,
                        lhsT=xT[:, ki, xTu + 128 * nt:xTu + 128 * nt + 128],
                        rhs=wT["Wv"][:, ki, :],
                        start=(ki == 0), stop=(ki == 3),
                    )
                nc.vector.tensor_copy(out=vv[:, nt, :], in_=psv)

            # ---- attention chunk loop: 16 chunks of 128 k' rows
            po = ps_o.tile([128, 4, 256], F32, tag="po")
            for kc, (jb, h) in enumerate(_CHUNK_ORDER):
                b0 = jb % 2
                lhsT_s = kt[64 * b0:64 * b0 + 64, jb // 2, 128 * h:128 * h + 128]
                pT = ppool.tile([128, 4, 2, 256], F32, tag="pT")
                rs = []
                for half in range(2):
                    pss = ps_s.tile([128, 4, 256], F32, tag="ps_s")
                    for jq4 in range(4):
                        jq = half * 4 + jq4
                        nc.tensor.matmul(
                            pss[:, jq4, :],
                            lhsT=lhsT_s,
                            rhs=qt[64 * b0:64 * b0 + 64, jq, :],
                            start=True, stop=True,
                            tile_position=(64 * b0, 0),
                        )
                    r = stats.tile([128, 1], F32, tag="rs")
                    nc.scalar.activation(
                        out=pT[:, 2 * half:2 * half + 2, :, :],
                        in_=pss, func=EXP, scale=0.125, accum_out=r,
                    )
                    rs.append(r)
                rz = stats.tile([128, 1], F32, tag="rz")
                nc.vector.tensor_add(out=rz, in0=rs[0], in1=rs[1])
                nc.vector.reciprocal(out=rz, in_=rz)
                vs = stats.tile([128, 64], F32, tag="vs")
                nc.vector.tensor_scalar_mul(
                    out=vs, in0=vv[:, h, 64 * jb:64 * jb + 64], scalar1=rz
                )
                st, sp = (kc == 0), (kc == 15)
                for e in range(2):
                    nc.tensor.matmul(
                        po[0:64, 2 * e:2 * e + 2, :],
                        lhsT=vs, rhs=pT[:, 2 * e:2 * e + 2, 0, :],
                        start=st, stop=sp, tile_position=(0, 0),
                    )
                    nc.tensor.matmul(
                        po[64:128, 2 * e:2 * e + 2, :],
                        lhsT=vs, rhs=pT[:, 2 * e:2 * e + 2, 1, :],
                        start=st, stop=sp, tile_position=(0, 64),
                    )

            # ---- OC^T evacuation (aligned) + final projection + bias
            ot = unitp.tile([128, 4, 256], F32, tag="ot")
            for i in range(4):
                nc.vector.tensor_copy(out=ot[:, i, :], in_=po[:, i, :])
            for m in range(2):
                psf = ps_m.tile([128, 512], F32, tag="misc")
                for ki in range(4):
                    nc.tensor.matmul(
                        psf,
                        lhsT=ot[:, ki, 128 * m:128 * m + 128],
                        rhs=wT["Wo"][:, ki, :],
                        start=(ki == 0), stop=(ki == 3),
                    )
                osb = outp.tile([128, D], F32, tag="osb")
                nc.vector.tensor_add(out=osb, in0=psf, in1=bo_bc)
                row = 256 * u + 128 * m
                nc.sync.dma_start(out=out_d[row:row + 128, :], in_=osb)

    nc.compile()
    return nc


def _get_nc():
    global _nc_cache
    if _nc_cache is None:
        _nc_cache = _build_nc()
    return _nc_cache


def _run(inputs, trace=False):
    from concourse.bass_utils import run_bass_kernel_spmd

    emb = np.ascontiguousarray(np.asarray(inputs["embedding"], dtype=np.float32))
    x_flat = emb.reshape(NCORES * TCORE, D)
    shared = {
        nm: np.ascontiguousarray(np.asarray(inputs[nm], dtype=np.float32))
        for nm in ("Wq", "Wk", "Wv", "Wo", "bo")
    }
    in_maps = []
    for c in range(NCORES):
        m = {"x": np.ascontiguousarray(x_flat[TCORE * c:TCORE * (c + 1)])}
        m.update(shared)
        in_maps.append(m)

    nc = _get_nc()
    res = run_bass_kernel_spmd(
        nc, in_maps, core_ids=list(range(NCORES)), trace=trace
    )
    out_flat = np.concatenate([r["out"] for r in res.results], axis=0)
    out = out_flat.reshape(emb.shape)
    return out, res


def kernel(**inputs):
    out, _ = _run(inputs, trace=False)
    return out


def _make_in_maps(inputs):
    emb = np.ascontiguousarray(np.asarray(inputs["embedding"], dtype=np.float32))
    x_flat = emb.reshape(NCORES * TCORE, D)
    shared = {
        nm: np.ascontiguousarray(np.asarray(inputs[nm], dtype=np.float32))
        for nm in ("Wq", "Wk", "Wv", "Wo", "bo")
    }
    in_maps = []
    for c in range(NCORES):
        m = {"x": np.ascontiguousarray(x_flat[TCORE * c:TCORE * (c + 1)])}
        m.update(shared)
        in_maps.append(m)
    return in_maps


def bench(inputs, iters=20):
    """Wall-clock the sharded PJRT executable; returns min per-iter ns.

    Mirrors run_bass_via_pjrt but keeps the jitted fn + device inputs so
    repeated calls time only the NEFF execution + dispatch overhead.
    """
    import time

    import jax
    import concourse.mybir as mybir
    from jax.sharding import Mesh, PartitionSpec
    from jax.experimental.shard_map import shard_map
    from concourse.bass2jax import (
        _bass_exec_p,
        install_neuronx_cc_hook,
        partition_id_tensor,
    )

    install_neuronx_cc_hook()
    nc = _get_nc()
    in_maps = _make_in_maps(inputs)

    partition_name = nc.partition_id_tensor.name if nc.partition_id_tensor else None
    in_names, out_names, out_avals, zero_outs = [], [], [], []
    for alloc in nc.m.functions[0].allocations:
        if not isinstance(alloc, mybir.MemoryLocationSet):
            continue
        name = alloc.memorylocations[0].name
        if alloc.kind == "ExternalInput":
            if name != partition_name:
                in_names.append(name)
        elif alloc.kind == "ExternalOutput":
            shape = tuple(alloc.tensor_shape)
            dtype = mybir.dt.np(alloc.dtype)
            out_names.append(name)
            out_avals.append(jax.core.ShapedArray(shape, dtype))
            zero_outs.append(np.zeros(shape, dtype))
    n_params = len(in_names)
    n_outs = len(out_avals)
    all_in_names = list(in_names) + list(out_names)
    if partition_name is not None:
        all_in_names.append(partition_name)

    def _body(*args):
        operands = list(args)
        if partition_name is not None:
            operands.append(partition_id_tensor())
        outs = _bass_exec_p.bind(
            *operands,
            out_avals=tuple(out_avals),
            in_names=tuple(all_in_names),
            out_names=tuple(out_names),
            lowering_input_output_aliases=(),
            sim_require_finite=True,
            sim_require_nnan=True,
            nc=nc,
        )
        return tuple(outs)

    devices = jax.devices()[:NCORES]
    mesh = Mesh(np.asarray(devices), ("core",))
    in_specs = (PartitionSpec("core"),) * (n_params + n_outs)
    out_specs = (PartitionSpec("core"),) * len(out_names)
    sharded = jax.jit(
        shard_map(_body, mesh=mesh, in_specs=in_specs, out_specs=out_specs,
                  check_rep=False),
        keep_unused=True,
    )
    per_core = [[np.asarray(m[nm]) for nm in in_names] for m in in_maps]
    concat_in = [
        np.concatenate([per_core[c][i] for c in range(NCORES)], axis=0)
        for i in range(n_params)
    ]
    concat_zeros = [
        np.zeros((NCORES * z.shape[0], *z.shape[1:]), z.dtype) for z in zero_outs
    ]
    args = [jax.device_put(a) for a in concat_in + concat_zeros]
    out = sharded(*args)
    jax.block_until_ready(out)
    times = []
    for _ in range(iters):
        t0 = time.perf_counter()
        out = sharded(*args)
        jax.block_until_ready(out)
        times.append(time.perf_counter() - t0)
    times.sort()
    print(f"bench: min {times[0]*1e6:.0f}us  p50 {times[len(times)//2]*1e6:.0f}us  "
          f"max {times[-1]*1e6:.0f}us over {iters} iters")
    return times[0] * 1e9
